# revision 3
# baseline (speedup 1.0000x reference)
# Bidirectional Mamba block on 8 TRN2 NeuronCores — v2.
#
# Sharding: core c = (b, dir, half): b = c // 4, dir = (c % 4) // 2,
# half = c % 2.  Each core runs one direction of one batch element for half
# (512) of the d_inner channels, BUT computes the in_proj/conv/silu (u) for
# ALL 1024 channels so the x-projection needs no cross-core AllReduce (the
# pair AllReduce in v1 cost ~55 us of exposed latency).  The only collective
# is the final 4-way ReduceScatter of the output projection partials.
#
# Other changes vs v1:
#  - all matmul operands bf16 (fp32 accumulation in PSUM)
#  - silu via AF.Silu directly from PSUM (no sigmoid+mult on DVE)
#  - selective-scan states merged Q at a time into one tensor_tensor_scan
#    (boundary reset via dA[:, seg_start] = 0), amortizing the scan's
#    per-instruction overhead
#  - the w = du*B broadcast-multiplies run on GpSimd (Pool) to free the DVE
#  - B/C broadcasts are SBUF->SBUF DMAs (no HBM round trip)
import time
import numpy as np
from contextlib import ExitStack

import concourse.bass as bass
import concourse.mybir as mybir
import concourse.tile as tile
from concourse import bass_utils

F32 = mybir.dt.float32
BF16 = mybir.dt.bfloat16
I32 = mybir.dt.int32
AF = mybir.ActivationFunctionType
OP = mybir.AluOpType

B, L, D = 2, 2048, 512
DI, DS, DTR, DCONV = 1024, 16, 32, 4
NCORE = 8
DH = DI // 2            # d_inner channels per core (own half)
NGF = DI // 128         # 8 channel groups of 128 (full)
NG = DH // 128          # 4 own channel groups
NT = L // 128           # 16 token tiles
NC512 = L // 512        # 4 chunks of 512 along t

Q = 2                   # states merged per scan instruction
NSG = DS // Q           # state groups per channel group
W_DVE_EVERY = 1         # every k-th w-multiply runs on DVE, rest on Pool

from kernel import _legalize_waits  # noqa: E402  (same wait-splitting pass)


def _build_nc(for_timeline=False):
    nc = bass.Bass("TRN2", target_bir_lowering=False, debug=False,
                   num_devices=NCORE)

    # ---------------- I/O declarations (per core) ----------------
    xpad_d = nc.dram_tensor("xpad", [D, DCONV - 1 + L], BF16,
                            kind="ExternalInput")
    winT_d = nc.dram_tensor("winT", [D, DI], BF16, kind="ExternalInput")
    wz_d = nc.dram_tensor("wz", [D, DH], BF16, kind="ExternalInput")
    wx_d = nc.dram_tensor("wx", [128, NGF * (DTR + 2 * DS)], BF16,
                          kind="ExternalInput")
    wdt_d = nc.dram_tensor("wdt", [DTR, DH], BF16, kind="ExternalInput")
    wout_d = nc.dram_tensor("wout", [128, NG * D], BF16,
                            kind="ExternalInput")
    consts_d = nc.dram_tensor("consts", [128, 128], F32,
                              kind="ExternalInput")
    xres_d = nc.dram_tensor("xres", [L // 4, D], F32, kind="ExternalInput")
    ln_g_d = nc.dram_tensor("ln_g", [128, D], F32, kind="ExternalInput")
    ln_b_d = nc.dram_tensor("ln_b", [128, D], F32, kind="ExternalInput")
    sidx_d = nc.dram_tensor("sidx", [128, NT], I32, kind="ExternalInput")
    ident_d = nc.dram_tensor("ident", [128, 128], BF16,
                             kind="ExternalInput")
    out_d = nc.dram_tensor("out_shard", [L // 4, D], F32,
                           kind="ExternalOutput")

    quad_groups = [[0, 1, 2, 3], [4, 5, 6, 7]]
    NPROJ = DTR + 2 * DS

    with tile.TileContext(nc) as tc:
        with ExitStack() as ctx:
            per = ctx.enter_context(tc.tile_pool(name="per", bufs=1))
            dram = ctx.enter_context(tc.tile_pool(name="dram", bufs=1,
                                                  space="DRAM"))

            out_bounce = dram.tile([L, D], BF16, tag="out_bounce",
                                   name="out_bounce")
            rs_out = dram.tile([L // 4, D], BF16, tag="rs_out",
                               name="rs_out")

            # packed constants: [0:32 convw(8g x 4)][32:40 convb]
            # [40:44 b_dt][44:108 A][108:112 dskip][112:113 eps]
            cst = per.tile([128, 128], F32, tag="cst", name="cst")
            nc.sync.dma_start(cst[:], consts_d.ap())
            convw = cst[:, 0:32]
            convb = cst[:, 32:40]
            b_dt_sb = cst[:, 40:44]
            A_sb = cst[:, 44:108]
            dskip_sb = cst[:, 108:112]
            eps_sb = cst[:, 112:113]
            sidx_sb = per.tile([128, NT], I32, tag="sidx", name="sidx")
            nc.sync.dma_start(sidx_sb[:], sidx_d.ap())

            # persistent activations (zs is spilled to DRAM until Phase F)
            u = [per.tile([128, L], BF16, tag=f"u{m}", name=f"u{m}")
                 for m in range(NGF)]
            dt = [per.tile([128, L], BF16, tag=f"dt{g}", name=f"dt{g}")
                  for g in range(NG)]
            du = [per.tile([128, L], BF16, tag=f"du{g}", name=f"du{g}")
                  for g in range(NG)]
            # sk holds the D_skip*u term during E; the PE fold result is
            # copied back over it afterwards (acc aliases sk)
            sk = [per.tile([128, L], BF16, tag=f"sk{g}", name=f"sk{g}")
                  for g in range(NG)]
            projT = per.tile([NPROJ, L], BF16, tag="projT", name="projT")
            ident = per.tile([128, 128], BF16, tag="ident", name="ident")
            nc.sync.dma_start(ident[:], ident_d.ap())
            wout_sb = per.tile([128, NG * D], BF16, tag="wout",
                               name="wout")
            nc.sync.dma_start(wout_sb[:], wout_d.ap())
            y_mm = [per.tile([128, L], BF16, tag=f"ymm{g}",
                             name=f"ymm{g}") for g in range(NG)]
            zs_dram = dram.tile([NG * 128, L], BF16, tag="zs_dram",
                                name="zs_dram")
            bc_dram = dram.tile([2 * DS, L], BF16, tag="bc_dram",
                                name="bc_dram")

            # ---------------- Phase A: in_proj + conv + silu ----------
            with ExitStack() as actx:
                pha = actx.enter_context(tc.tile_pool(name="pha", bufs=1))
                scr = actx.enter_context(tc.tile_pool(name="scr", bufs=2))
                psA = actx.enter_context(
                    tc.tile_pool(name="psA", bufs=4, space="PSUM"))
                xpad_sb = [pha.tile([128, DCONV - 1 + L], BF16,
                                    tag=f"xp{k}", name=f"xp{k}")
                           for k in range(4)]
                winT_sb = [pha.tile([128, DI], BF16, tag=f"wi{k}",
                                    name=f"wi{k}") for k in range(4)]
                wz_sb = [pha.tile([128, DH], BF16, tag=f"wzk{k}",
                                  name=f"wzk{k}") for k in range(4)]
                for k in range(4):
                    sl = slice(k * 128, (k + 1) * 128)
                    nc.sync.dma_start(xpad_sb[k][:], xpad_d.ap()[sl, :])
                    nc.sync.dma_start(winT_sb[k][:], winT_d.ap()[sl, :])
                    nc.sync.dma_start(wz_sb[k][:], wz_d.ap()[sl, :])

                xc_pre = [pha.tile([128, L], BF16, tag=f"xcp{m}",
                                   name=f"xcp{m}") for m in range(NGF)]
                # xc for the FULL d_inner
                for m in range(NGF):
                    for nn in range(NC512):
                        ps = psA.tile([128, 512], F32, tag="mmA", name="mmA")
                        for k in range(4):
                            nc.tensor.matmul(
                                out=ps[:],
                                lhsT=winT_sb[k][:, m * 128:(m + 1) * 128],
                                rhs=xpad_sb[k][:, DCONV - 1 + nn * 512:
                                               DCONV - 1 + (nn + 1) * 512],
                                start=(k == 0), stop=(k == 3))
                        nc.scalar.copy(
                            xc_pre[m][:, nn * 512:(nn + 1) * 512], ps[:])
                # z (own half) -> silu -> zs (spilled to DRAM)
                for g in range(NG):
                    zt = scr.tile([128, L], BF16, tag="zt", name="zt")
                    for nn in range(NC512):
                        ps = psA.tile([128, 512], F32, tag="mmA", name="mmA")
                        for k in range(4):
                            nc.tensor.matmul(
                                out=ps[:],
                                lhsT=wz_sb[k][:, g * 128:(g + 1) * 128],
                                rhs=xpad_sb[k][:, DCONV - 1 + nn * 512:
                                               DCONV - 1 + (nn + 1) * 512],
                                start=(k == 0), stop=(k == 3))
                        nc.scalar.activation(
                            zt[:, nn * 512:(nn + 1) * 512], ps[:],
                            AF.Silu)
                    nc.sync.dma_start(
                        zs_dram[g * 128:(g + 1) * 128, :], zt[:])
                # causal conv (bf16) + silu -> u   (all 8 groups)
                # u[t] = silu(conv_b + sum_j w[3-j] * xc[t-j])
                for m in range(NGF):
                    a_t = scr.tile([128, L], BF16, tag="cacc", name="cacc")
                    nc.vector.tensor_scalar(
                        a_t[:], xc_pre[m][:],
                        convw[:, m * DCONV + DCONV - 1:m * DCONV + DCONV],
                        convb[:, m:m + 1], OP.mult, OP.add)
                    for j in range(1, DCONV):
                        nc.vector.scalar_tensor_tensor(
                            a_t[:, j:L], xc_pre[m][:, 0:L - j],
                            convw[:, m * DCONV + DCONV - 1 - j:
                                  m * DCONV + DCONV - j],
                            a_t[:, j:L], OP.mult, OP.add)
                    nc.scalar.activation(u[m][:], a_t[:], AF.Silu)

            # ---------------- Phase C: x-projection (no collective) ----
            with ExitStack() as cctx:
                phc = cctx.enter_context(tc.tile_pool(name="phc", bufs=1))
                psC = cctx.enter_context(
                    tc.tile_pool(name="psC", bufs=2, space="PSUM"))
                wx_sb = phc.tile([128, NGF * NPROJ], BF16, tag="wx",
                                 name="wx")
                nc.sync.dma_start(wx_sb[:], wx_d.ap())
                for nn in range(NC512):
                    ps = psC.tile([NPROJ, 512], F32, tag="mmC", name="mmC")
                    for kg in range(NGF):
                        nc.tensor.matmul(
                            out=ps[:],
                            lhsT=wx_sb[:, kg * NPROJ:(kg + 1) * NPROJ],
                            rhs=u[kg][:, nn * 512:(nn + 1) * 512],
                            start=(kg == 0), stop=(kg == NGF - 1))
                    nc.scalar.copy(projT[:, nn * 512:(nn + 1) * 512], ps[:])
                    nc.sync.dma_start(
                        bc_dram[:, nn * 512:(nn + 1) * 512],
                        projT[DTR:NPROJ, nn * 512:(nn + 1) * 512])

            # ---------------- Phase D: dt = softplus(...), du ----------
            with ExitStack() as dctx:
                psD = dctx.enter_context(
                    tc.tile_pool(name="psD", bufs=2, space="PSUM"))
                phd = dctx.enter_context(tc.tile_pool(name="phd", bufs=2))
                wdt_sb = phd.tile([DTR, DH], BF16, tag="wdt", name="wdt",
                                  bufs=1)
                nc.sync.dma_start(wdt_sb[:], wdt_d.ap())
                for nn in range(NC512):
                    for g in range(NG):
                        ps = psD.tile([128, 512], F32, tag="mmD", name="mmD")
                        nc.tensor.matmul(
                            out=ps[:],
                            lhsT=wdt_sb[:, g * 128:(g + 1) * 128],
                            rhs=projT[0:DTR, nn * 512:(nn + 1) * 512],
                            start=True, stop=True)
                        edt = phd.tile([128, 512], F32, tag="edt",
                                       name="edt")
                        nc.scalar.activation(edt[:], ps[:], AF.Exp,
                                             bias=b_dt_sb[:, g:g + 1])
                        nc.scalar.activation(
                            dt[g][:, nn * 512:(nn + 1) * 512], edt[:],
                            AF.Ln, bias=1.0)
                # host-side channel perm puts this core's own half at
                # groups 0..NG-1 of the full-u layout
                for g in range(NG):
                    nc.vector.tensor_tensor(out=du[g][:], in0=dt[g][:],
                                            in1=u[g][:], op=OP.mult)
                    nc.vector.tensor_scalar(
                        sk[g][:], u[g][:], dskip_sb[:, g:g + 1], None,
                        OP.mult)

            # ---------------- Phase E: selective scan ----------------
            # Two passes of 2 channel groups; per pass the 16 states run in
            # NSG merged-scan groups of Q.  The sum over states (and the
            # D_skip*u term) accumulates on the PE via identity matmuls
            # into PSUM (4 banks per group).
            wcnt = [0]
            with ExitStack() as ectx:
                trb = ectx.enter_context(tc.tile_pool(name="trb", bufs=2))
                tr1 = ectx.enter_context(tc.tile_pool(name="tr1", bufs=1))
                psY = ectx.enter_context(
                    tc.tile_pool(name="psY", bufs=1, space="PSUM"))
                for gp in range(NG // 2):
                    gs = [2 * gp, 2 * gp + 1]
                    ytiles = {g: [psY.tile([128, 512], F32,
                                           tag=f"psY{g % 2}_{nn}",
                                           name=f"psY{g}_{nn}")
                                  for nn in range(NC512)] for g in gs}
                    for sg in range(NSG):
                        n0 = sg * Q
                        Bq = trb.tile([128, Q * L], BF16, tag="Bq",
                                      name="Bq")
                        Cq = trb.tile([128, Q * L], BF16, tag="Cq",
                                      name="Cq")
                        for q in range(Q):
                            nrow = n0 + q
                            nc.sync.dma_start(
                                Bq[:, q * L:(q + 1) * L],
                                bc_dram[nrow:nrow + 1, :].to_broadcast(
                                    [128, L]))
                            nc.sync.dma_start(
                                Cq[:, q * L:(q + 1) * L],
                                bc_dram[DS + nrow:DS + nrow + 1,
                                        :].to_broadcast([128, L]))
                        for g in gs:
                            dA = trb.tile([128, Q * L], BF16, tag="dA",
                                          name="dA")
                            for q in range(Q):
                                nidx = g * DS + n0 + q
                                nc.scalar.activation(
                                    dA[:, q * L + 1:(q + 1) * L],
                                    dt[g][:, 1:L],
                                    AF.Exp, scale=A_sb[:, nidx:nidx + 1])
                                nc.vector.memset(dA[:, q * L:q * L + 1],
                                                 0.0)
                            w_t = trb.tile([128, Q * L], BF16, tag="w",
                                           name="w")
                            for q in range(Q):
                                wcnt[0] += 1
                                if wcnt[0] % W_DVE_EVERY == 0:
                                    nc.vector.tensor_tensor(
                                        out=w_t[:, q * L:(q + 1) * L],
                                        in0=du[g][:],
                                        in1=Bq[:, q * L:(q + 1) * L],
                                        op=OP.mult)
                                else:
                                    nc.gpsimd.tensor_tensor(
                                        out=w_t[:, q * L:(q + 1) * L],
                                        in0=du[g][:],
                                        in1=Bq[:, q * L:(q + 1) * L],
                                        op=OP.mult)
                            h_t = trb.tile([128, Q * L], BF16, tag="h",
                                           name="h")
                            nc.vector.tensor_tensor_scan(
                                h_t[:], dA[:], w_t[:], 0.0, OP.mult,
                                OP.add)
                            p_t = trb.tile([128, Q * L], BF16, tag="p",
                                           name="p")
                            nc.vector.tensor_tensor(out=p_t[:], in0=h_t[:],
                                                    in1=Cq[:], op=OP.mult)
                            first = (sg == 0)
                            last = (sg == NSG - 1)
                            for nn in range(NC512):
                                if first:
                                    # chain starts with the D_skip*u term
                                    nc.tensor.matmul(
                                        out=ytiles[g][nn][:],
                                        lhsT=ident[:],
                                        rhs=sk[g][:, nn * 512:
                                                  (nn + 1) * 512],
                                        start=True, stop=False)
                                for q in range(Q):
                                    nc.tensor.matmul(
                                        out=ytiles[g][nn][:],
                                        lhsT=ident[:],
                                        rhs=p_t[:, q * L + nn * 512:
                                                q * L + (nn + 1) * 512],
                                        start=False,
                                        stop=(last and q == Q - 1))
                    for g in gs:
                        for nn in range(NC512):
                            nc.scalar.copy(
                                sk[g][:, nn * 512:(nn + 1) * 512],
                                ytiles[g][nn][:])
                        # gate with silu(z) as soon as this pass's fold
                        # lands, overlapping the other pass's scans
                        zt = trb.tile([128, L], BF16, tag="zt2",
                                      name="zt2", bufs=1)
                        nc.sync.dma_start(
                            zt[:], zs_dram[g * 128:(g + 1) * 128, :])
                        nc.vector.tensor_tensor(out=y_mm[g][:],
                                                in0=sk[g][:],
                                                in1=zt[:], op=OP.mult)

            # ---------------- Phase F: out_proj ----------------
            with ExitStack() as fctx:
                psF = fctx.enter_context(
                    tc.tile_pool(name="psF", bufs=4, space="PSUM"))
                osb_pool = fctx.enter_context(tc.tile_pool(name="osb",
                                                           bufs=4))
                for tt in range(NT):
                    ps = psF.tile([128, D], F32, tag="mmF", name="mmF")
                    for g in range(NG):
                        nc.tensor.matmul(
                            out=ps[:],
                            lhsT=y_mm[g][:, tt * 128:(tt + 1) * 128],
                            rhs=wout_sb[:, g * D:(g + 1) * D],
                            start=(g == 0), stop=(g == NG - 1))
                    o_sb = osb_pool.tile([128, D], BF16, tag="osb",
                                         name="osb")
                    nc.scalar.copy(o_sb[:], ps[:])
                    nc.gpsimd.indirect_dma_start(
                        out=out_bounce.opt(),
                        out_offset=bass.IndirectOffsetOnAxis(
                            ap=sidx_sb[:, tt:tt + 1], axis=0),
                        in_=o_sb[:],
                        in_offset=None)

            # -------- Phase G: ReduceScatter + LN/SiLU/residual --------
            with ExitStack() as gctx:
                phg = gctx.enter_context(tc.tile_pool(name="phg", bufs=1))
                # prefetch the residual/LN params before the collective so
                # their DMAs overlap the ReduceScatter wait
                xres_sb = phg.tile([128, 4 * D], F32, tag="xres",
                                   name="xres")
                for qq in range(4):
                    nc.sync.dma_start(xres_sb[:, qq * D:(qq + 1) * D],
                                      xres_d.ap()[qq * 128:(qq + 1) * 128,
                                                  :])
                lng_sb = phg.tile([128, D], F32, tag="lng", name="lng")
                nc.sync.dma_start(lng_sb[:], ln_g_d.ap())
                lnb_sb = phg.tile([128, D], F32, tag="lnb", name="lnb")
                nc.sync.dma_start(lnb_sb[:], ln_b_d.ap())

                if for_timeline:
                    nc.sync.dma_start(rs_out[:], out_bounce[0:L // 4, :])
                else:
                    nc.gpsimd.collective_compute(
                        "ReduceScatter", OP.add, replica_groups=quad_groups,
                        ins=[out_bounce.opt()], outs=[rs_out.opt()])
                rs_bf = phg.tile([128, 4 * D], BF16, tag="rsb", name="rsb")
                rs_sb = phg.tile([128, 4 * D], F32, tag="rs", name="rs")
                for qq in range(4):
                    nc.sync.dma_start(rs_bf[:, qq * D:(qq + 1) * D],
                                      rs_out[qq * 128:(qq + 1) * 128, :])
                nc.vector.tensor_copy(rs_sb[:], rs_bf[:])

                for qq in range(4):
                    rtile = rs_sb[:, qq * D:(qq + 1) * D]
                    mu = phg.tile([128, 1], F32, tag=f"mu{qq}",
                                  name=f"mu{qq}")
                    nc.vector.tensor_reduce(mu[:], rtile,
                                            mybir.AxisListType.X, OP.add)
                    negmu = phg.tile([128, 1], F32, tag=f"negmu{qq}",
                                     name=f"negmu{qq}")
                    nc.vector.tensor_scalar(negmu[:], mu[:], -1.0 / D,
                                            None, OP.mult)
                    xm = phg.tile([128, D], F32, tag=f"xm{qq}",
                                  name=f"xm{qq}")
                    nc.scalar.activation(xm[:], rtile, AF.Identity,
                                         bias=negmu[:, 0:1])
                    ss = phg.tile([128, 1], F32, tag=f"ss{qq}",
                                  name=f"ss{qq}")
                    sq = phg.tile([128, D], F32, tag=f"sq{qq}",
                                  name=f"sq{qq}")
                    nc.scalar.activation(sq[:], xm[:], AF.Square,
                                         accum_out=ss[:])
                    std = phg.tile([128, 1], F32, tag=f"std{qq}",
                                   name=f"std{qq}")
                    nc.scalar.activation(std[:], ss[:], AF.Sqrt,
                                         bias=eps_sb[:, 0:1], scale=1.0 / D)
                    rstd = phg.tile([128, 1], F32, tag=f"rstd{qq}",
                                    name=f"rstd{qq}")
                    nc.vector.reciprocal(rstd[:], std[:])
                    s1 = phg.tile([128, D], F32, tag=f"s1{qq}",
                                  name=f"s1{qq}")
                    nc.scalar.activation(s1[:], xm[:], AF.Identity,
                                         scale=rstd[:, 0:1])
                    s2 = phg.tile([128, D], F32, tag=f"s2{qq}",
                                  name=f"s2{qq}")
                    nc.vector.tensor_tensor(out=s2[:], in0=s1[:],
                                            in1=lng_sb[:], op=OP.mult)
                    s3 = phg.tile([128, D], F32, tag=f"s3{qq}",
                                  name=f"s3{qq}")
                    nc.vector.tensor_tensor(out=s3[:], in0=s2[:],
                                            in1=lnb_sb[:], op=OP.add)
                    sil = phg.tile([128, D], F32, tag=f"sil{qq}",
                                   name=f"sil{qq}")
                    nc.scalar.activation(sil[:], s3[:], AF.Silu)
                    fin = phg.tile([128, D], F32, tag=f"fin{qq}",
                                   name=f"fin{qq}")
                    nc.vector.tensor_tensor(
                        out=fin[:], in0=sil[:],
                        in1=xres_sb[:, qq * D:(qq + 1) * D], op=OP.add)
                    nc.sync.dma_start(
                        out_d.ap()[qq * 128:(qq + 1) * 128, :], fin[:])

    _legalize_waits(nc)
    return nc
